# revision 6
# baseline (speedup 1.0000x reference)
"""Trainium2 Bass kernel for nn_CrossAttentionProjectLayer.

Reference computation (per batch b, head h):
  pk = enc @ k_w.T + k_b            [S, E] -> [S, H, D]
  pv = enc @ v_w.T + v_b            [S, H, D]
  rm = sigma * random_matrices      [H, K, D]
  proj = pk @ rm_h.T                [S, H, K]
  phi = [sin(proj), cos(proj)] * K^-0.5, masked per token   [S, H, 2K]
  s = sum_S phi^T pv                [H, 2K, D]
  z = sum_S phi                     [H, 2K]

Strategy: batch (B=8) data-parallel across the 8 NeuronCores. On host,
fold rm into k_w (W2 = rm @ k_w_head) so the device computes
u = enc @ W2'.T + b2' directly with W2' prescaled by 1/(2 pi) (u = proj/2pi).
sin/cos evaluated via range reduction (ACT round-to-int cast + DVE subtract,
then ScalarE Sin over [-pi, pi]). The S-reduction accumulates in PSUM over
all 32 s-tiles per head: out[k2, 0:64] = s, out[k2, 64] = z (ones column of
the moving operand carries the mask coefficient, so z rides along free).
"""

import numpy as np

S, B, E = 4096, 8, 1024
H, D, K = 16, 64, 64
K2 = 2 * K
TAU = 1.0
NCORES = 8
SC = 512            # s-chunk (DMA granularity)
NCHUNK = S // SC    # 8
NSUB = SC // 128    # 4 s-subtiles per chunk
NST = S // 128      # 32 s-tiles
EI = E // 128       # 8 contraction tiles

_CACHE = {}


def _build_nc(s_len=S, mm_dt_name="float32r", p3_dt_name="float16"):
    import concourse.bass as bass  # noqa: F401
    import concourse.mybir as mybir
    import concourse.tile as tile
    from concourse import bacc

    dt = mybir.dt
    mm_dt = getattr(dt, mm_dt_name)
    p3_dt = getattr(dt, p3_dt_name)
    AF = mybir.ActivationFunctionType
    ALU = mybir.AluOpType

    nst = s_len // 128
    nchunk = max(1, s_len // SC)
    sc = s_len // nchunk
    nsub = sc // 128

    nc = bacc.Bacc("TRN2", target_bir_lowering=False, debug=False,
                   num_devices=NCORES)

    enc_d = nc.dram_tensor("encT", [E, s_len], mm_dt, kind="ExternalInput")
    w2t_d = nc.dram_tensor("w2t", [E, E], mm_dt, kind="ExternalInput")
    vwt_d = nc.dram_tensor("vwt", [E, E], mm_dt, kind="ExternalInput")
    b2r_d = nc.dram_tensor("b2row", [1, E], mm_dt, kind="ExternalInput")
    vbr_d = nc.dram_tensor("vbrow", [1, E], mm_dt, kind="ExternalInput")
    mkf_d = nc.dram_tensor("maskf", [s_len], dt.float32, kind="ExternalInput")
    outs_d = nc.dram_tensor("out_s", [H, K2, D], dt.float32, kind="ExternalOutput")
    outz_d = nc.dram_tensor("out_z", [K2, H], dt.float32, kind="ExternalOutput")

    TWO_PI = float(2 * np.pi)
    HALF_PI = float(np.pi / 2)

    with tile.TileContext(nc) as tc:
        with (
            tc.tile_pool(name="wpool", bufs=1) as wpool,
            tc.tile_pool(name="cpool", bufs=2) as cpool,
            tc.tile_pool(name="npool", bufs=4) as npool,
            tc.tile_pool(name="rpool", bufs=4) as rpool,
            tc.tile_pool(name="phipool", bufs=2) as phipool,
            tc.tile_pool(name="pvmpool", bufs=2) as pvmpool,
            tc.tile_pool(name="spool", bufs=1) as spool,
            tc.tile_pool(name="psum", bufs=4, space="PSUM") as psum,
            tc.tile_pool(name="accp", bufs=1, space="PSUM") as accp,
        ):
            # ---- persistent weights / constants ----
            w2t_sb = wpool.tile([128, EI, E], mm_dt)
            vwt_sb = wpool.tile([128, EI, E], mm_dt)
            w2t_ap = w2t_d[:].rearrange("(a p) n -> p a n", p=128)
            vwt_ap = vwt_d[:].rearrange("(a p) n -> p a n", p=128)
            for ei in range(EI):
                nc.sync.dma_start(out=w2t_sb[:, ei, :], in_=w2t_ap[:, ei, :])
                nc.sync.dma_start(out=vwt_sb[:, ei, :], in_=vwt_ap[:, ei, :])
            b2row = wpool.tile([1, E], mm_dt)
            vbrow = wpool.tile([1, E], mm_dt)
            nc.sync.dma_start(out=b2row[:], in_=b2r_d[:])
            nc.sync.dma_start(out=vbrow[:], in_=vbr_d[:])
            onesrow_f = wpool.tile([1, 128], dt.float32)
            nc.vector.memset(onesrow_f[:], 1.0)
            onesrow = onesrow_f[:].bitcast(mm_dt)
            pi_half = wpool.tile([128, 1], dt.float32)
            nc.vector.memset(pi_half[:], HALF_PI)
            mkf_sb = wpool.tile([128, nst], dt.float32)
            nc.sync.dma_start(
                out=mkf_sb[:], in_=mkf_d[:].rearrange("(t p) -> p t", p=128)
            )

            # ---- persistent PSUM accumulators: 4 banks x 4 heads x 65 ----
            acc_tiles = [accp.tile([128, 4 * 65], dt.float32, name=f"acc{j}")
                         for j in range(4)]

            enc_ap = enc_d[:].rearrange("(a p) n -> p a n", p=128)

            for ch in range(nchunk):
                enc_sb = cpool.tile([128, EI, sc], mm_dt, name="enc_sb")
                for ei in range(EI):
                    nc.sync.dma_start(
                        out=enc_sb[:, ei, :],
                        in_=enc_ap[:, ei, ch * sc:(ch + 1) * sc],
                    )
                for sub in range(nsub):
                    st = ch * nsub + sub
                    lhs_sl = slice(sub * 128, (sub + 1) * 128)
                    mcol = mkf_sb[:, st:st + 1]

                    phi_t = phipool.tile([128, H, K2], p3_dt, name="phi_t")
                    pvm_t = pvmpool.tile([128, H, 65], p3_dt, name="pvm_t")

                    for hf in range(2):
                        nsl = slice(hf * 512, (hf + 1) * 512)
                        hsl = slice(hf * 8, (hf + 1) * 8)
                        # ---- u = enc @ W2'(half) + b2' ----
                        ps = psum.tile([128, 512], dt.float32, name="ps", tag="pswork")
                        for ei in range(EI):
                            nc.tensor.matmul(
                                ps[:], enc_sb[:, ei, lhs_sl], w2t_sb[:, ei, nsl],
                                start=(ei == 0), stop=False,
                            )
                        nc.tensor.matmul(ps[:], onesrow, b2row[:, nsl],
                                         start=False, stop=True)
                        # ---- range reduce + sin/cos ----
                        ni = npool.tile([128, 512], dt.int32, name="ni")
                        nc.scalar.activation(ni[:], ps[:], AF.Copy)
                        ni2 = npool.tile([128, 512], dt.int32, name="ni2")
                        nc.scalar.activation(ni2[:], ps[:], AF.Copy, bias=0.25)
                        r1 = rpool.tile([128, 512], dt.float32, name="r1")
                        nc.vector.tensor_tensor(out=r1[:], in0=ps[:], in1=ni[:],
                                                op=ALU.subtract)
                        r2 = rpool.tile([128, 512], dt.float32, name="r2")
                        nc.vector.tensor_tensor(out=r2[:], in0=ps[:], in1=ni2[:],
                                                op=ALU.subtract)
                        nc.scalar.activation(
                            phi_t[:, hsl, 0:K],
                            r1[:].rearrange("p (h k) -> p h k", h=8),
                            AF.Sin, scale=TWO_PI,
                        )
                        nc.scalar.activation(
                            phi_t[:, hsl, K:K2],
                            r2[:].rearrange("p (h k) -> p h k", h=8),
                            AF.Sin, scale=TWO_PI, bias=pi_half[:],
                        )
                        # ---- pv half ----
                        ps2 = psum.tile([128, 512], dt.float32, name="ps2", tag="pswork")
                        for ei in range(EI):
                            nc.tensor.matmul(
                                ps2[:], enc_sb[:, ei, lhs_sl], vwt_sb[:, ei, nsl],
                                start=(ei == 0), stop=False,
                            )
                        nc.tensor.matmul(ps2[:], onesrow, vbrow[:, nsl],
                                         start=False, stop=True)
                        nc.vector.tensor_scalar(
                            out=pvm_t[:, hsl, 0:D],
                            in0=ps2[:].rearrange("p (h d) -> p h d", h=8),
                            scalar1=mcol, scalar2=None, op0=ALU.mult,
                        )
                    # mask coefficient into the ones-columns (-> z)
                    nc.vector.tensor_copy(pvm_t[:, :, 64],
                                          mcol.broadcast_to([128, H]))
                    # ---- phase 3: per-head phi^T @ [pvm | m] accumulate ----
                    for h in range(H):
                        acc_t = acc_tiles[h // 4]
                        csl = slice((h % 4) * 65, (h % 4) * 65 + 65)
                        nc.tensor.matmul(
                            acc_t[:, csl], phi_t[:, h, :], pvm_t[:, h, :],
                            start=(st == 0 and h % 4 == 0),
                            stop=(st == nst - 1 and h % 4 == 3),
                        )

            # ---- evacuate accumulators and store ----
            stage = spool.tile([128, 4 * 4 * 65], dt.float32)
            for j in range(4):
                nc.vector.tensor_copy(stage[:, j * 260:(j + 1) * 260],
                                      acc_tiles[j][:])
            stage3 = stage[:].rearrange("p (h c) -> p h c", c=65)
            nc.sync.dma_start(out=outs_d[:].rearrange("h p d -> p h d"),
                              in_=stage3[:, :, 0:D])
            nc.sync.dma_start(out=outz_d[:], in_=stage3[:, :, 64])

    nc.compile()
    return nc


def _host_prep(encoder_output, random_matrices, k_w, k_b, v_w, v_b, sigma, mask,
               s_len=S):
    f32 = np.float32
    enc = np.asarray(encoder_output, f32)[:s_len]
    rm = (np.asarray(sigma, f32) * np.asarray(random_matrices, f32)) / TAU
    k_w64 = np.asarray(k_w, np.float64).reshape(H, D, E)
    k_b64 = np.asarray(k_b, np.float64).reshape(H, D)
    rm64 = rm.astype(np.float64)
    inv2pi = 1.0 / (2 * np.pi)
    # W2[h,k,e] = sum_d rm[h,k,d] k_w[h*D+d, e]; prescaled by 1/(2pi)
    W2 = np.einsum("hkd,hde->hke", rm64, k_w64) * inv2pi
    b2 = np.einsum("hkd,hd->hk", rm64, k_b64) * inv2pi
    w2t = np.ascontiguousarray(W2.reshape(H * K, E).T.astype(f32))
    b2row = b2.reshape(1, H * K).astype(f32)
    vwt = np.ascontiguousarray(np.asarray(v_w, f32).T)
    vbrow = np.asarray(v_b, f32).reshape(1, E)
    encT = np.ascontiguousarray(enc.transpose(1, 2, 0))  # [B, E, s_len]
    m = np.asarray(mask).reshape(-1, B)[:s_len]
    maskf = np.where(m, f32(0.0), f32(K ** -0.5)).astype(f32)  # [s_len, B]
    in_maps = []
    for b in range(NCORES):
        in_maps.append({
            "encT": encT[b],
            "w2t": w2t,
            "vwt": vwt,
            "b2row": b2row,
            "vbrow": vbrow,
            "maskf": np.ascontiguousarray(maskf[:, b]),
        })
    return in_maps, rm.astype(f32)


def _run(inputs, s_len=S, trace=False, mm_dt_name="float32r",
         p3_dt_name="float16"):
    from concourse.bass_utils import run_bass_kernel_spmd

    key = (s_len, mm_dt_name, p3_dt_name)
    if key not in _CACHE:
        _CACHE[key] = _build_nc(s_len, mm_dt_name, p3_dt_name)
    nc = _CACHE[key]
    in_maps, rm = _host_prep(**inputs, s_len=s_len)
    res = run_bass_kernel_spmd(nc, in_maps, core_ids=list(range(NCORES)),
                               trace=trace)
    s_full = np.stack([res.results[b]["out_s"] for b in range(NCORES)])
    z_full = np.stack([res.results[b]["out_z"].T for b in range(NCORES)])
    return (s_full, z_full, rm), res


def kernel(**inputs):
    (s_full, z_full, rm), _ = _run(inputs)
    return s_full, z_full, rm
